# revision 1
# baseline (speedup 1.0000x reference)
"""DynamicSparseMoE grouped-GEMM kernel for 8 TRN2 NeuronCores.

out[t] = tokens[t] @ weight[exp_ids[t]]   (T=8192, E=8, D=2048 -> 2048)

Strategy (expert-parallel, host-side dispatch):
  - Host sorts tokens by expert; core e owns expert e's weight and its
    routed tokens, padded to a common capacity C (SPMD needs equal shapes).
  - Inputs are cast to fp16 on the host (PE runs fp16 at 1 cyc/row vs
    fp32's 4; PSUM accumulation stays fp32, measured rel-err ~3e-4).
  - Tokens are passed transposed ([D, C]) so matmul operands need no
    on-chip transpose: stationary = weight tile [d,128 o], moving =
    xT tile [d, t]; psum out = [o, t] accumulated over 16 d-blocks.
  - Weight (8 MB fp16) is resident in SBUF; xT streams per t-tile.
  - Output [2048, C] fp32 per core; host unpermutes back to [T, D].
"""

import numpy as np

P = 128
D = 2048
E = 8
KB = D // P  # 16 contraction blocks
OB = D // P  # 16 output blocks
NT = 512  # moving free dim per matmul (one fp32 PSUM bank)

_cache = {}


def _ensure_imports():
    try:
        import concourse.bass  # noqa: F401
    except ImportError:
        import sys

        for p in ("/opt/trn_rl_repo", "/opt/pypackages"):
            if p not in sys.path:
                sys.path.append(p)


def _build(C):
    """Build + compile the per-core Bass program for capacity C."""
    _ensure_imports()
    import concourse.bacc as bacc
    import concourse.mybir as mybir
    import concourse.tile as tile

    t_tiles = []
    t0 = 0
    while t0 < C:
        nt = min(NT, C - t0)
        t_tiles.append((t0, nt))
        t0 += nt

    nc = bacc.Bacc(None, target_bir_lowering=False, debug=False)
    xt_d = nc.declare_dram_parameter("xt", [D, C], mybir.dt.float16, isOutput=False)
    w_d = nc.declare_dram_parameter("w", [D, D], mybir.dt.float16, isOutput=False)
    out_d = nc.declare_dram_parameter("outT", [D, C], mybir.dt.float32, isOutput=True)

    xt_t = xt_d.rearrange("(k p) n -> p k n", p=P)  # [128, 16, C]
    w_t = w_d.rearrange("(k p) o -> p k o", p=P)  # [128, 16, 2048]

    with tile.TileContext(nc) as tc:
        with (
            tc.tile_pool(name="wp", bufs=1) as wp,
            tc.tile_pool(name="xp", bufs=3) as xp,
            tc.tile_pool(name="op", bufs=4) as op,
            tc.tile_pool(name="pp", bufs=4, space="PSUM") as pp,
        ):
            # Weight resident in SBUF: one tile per d-block so the first
            # matmuls can start before the whole 8 MB lands.
            w_sb = []
            for kb in range(KB):
                wt = wp.tile([P, D], mybir.dt.float16, tag=f"w{kb}")
                nc.sync.dma_start(wt[:], w_t[:, kb, :])
                w_sb.append(wt)

            for t0, nt in t_tiles:
                x_sb = xp.tile([P, KB * nt], mybir.dt.float16, tag="x")
                nc.sync.dma_start(
                    x_sb.rearrange("p (k n) -> p k n", k=KB),
                    xt_t[:, :, t0 : t0 + nt],
                )
                for ob in range(OB):
                    ps = pp.tile([P, nt], mybir.dt.float32, tag="ps")
                    for kb in range(KB):
                        nc.tensor.matmul(
                            ps[:],
                            lhsT=w_sb[kb][:, ob * P : (ob + 1) * P],
                            rhs=x_sb[:, kb * nt : (kb + 1) * nt],
                            start=(kb == 0),
                            stop=(kb == KB - 1),
                        )
                    o_sb = op.tile([P, nt], mybir.dt.float32, tag="o")
                    nc.vector.tensor_copy(o_sb[:], ps[:])
                    nc.sync.dma_start(out_d[ob * P : (ob + 1) * P, t0 : t0 + nt], o_sb[:])
    nc.compile()
    return nc


def _get_nc(C):
    if C not in _cache:
        _cache[C] = _build(C)
    return _cache[C]


def kernel(tokens, weight, exp_ids, _trace=False):
    _ensure_imports()
    from concourse.bass_utils import run_bass_kernel_spmd

    tokens = np.asarray(tokens)
    weight = np.asarray(weight)
    exp_ids = np.asarray(exp_ids)
    T = tokens.shape[0]

    order = np.argsort(exp_ids, kind="stable")
    counts = np.bincount(exp_ids, minlength=E)
    C = max(int(-(-counts.max() // P) * P), NT)

    starts = np.zeros(E + 1, dtype=np.int64)
    np.cumsum(counts, out=starts[1:])

    tokens16 = tokens.astype(np.float16)
    weight16 = weight.astype(np.float16)

    in_maps = []
    for e in range(E):
        idx = order[starts[e] : starts[e + 1]]
        xt = np.zeros((D, C), dtype=np.float16)
        xt[:, : counts[e]] = tokens16[idx].T
        in_maps.append({"xt": xt, "w": np.ascontiguousarray(weight16[e])})

    nc = _get_nc(C)
    res = run_bass_kernel_spmd(
        nc,
        in_maps,
        core_ids=list(range(E)),
        trace=_trace,
        trace_cores=list(range(E)) if _trace else None,
    )

    out = np.empty((T, D), dtype=np.float32)
    for e in range(E):
        idx = order[starts[e] : starts[e + 1]]
        out[idx] = res.results[e]["outT"][:, : counts[e]].T
    if _trace:
        return out, res
    return out


# revision 3
# speedup vs baseline: 1.1264x; 1.1264x over previous
"""DynamicSparseMoE grouped-GEMM kernel for 8 TRN2 NeuronCores.

out[t] = tokens[t] @ weight[exp_ids[t]]   (T=8192, E=8, D=2048 -> 2048)

Strategy (expert-parallel, host-side dispatch):
  - Host sorts tokens by expert; core e owns expert e's weight and its
    routed tokens, padded to a common capacity C (SPMD needs equal shapes).
  - Inputs are cast to fp16 on the host (PE runs fp16 at 1 cyc/row vs
    fp32's 4; PSUM accumulation stays fp32, measured rel-err ~3e-4).
  - Tokens are passed transposed ([D, C]): the stationary operand is a
    token block xT[d-block, 128 t] (one LDWEIGHTS per 4 matmuls), the
    moving operand is a weight slice w[d-block, 512 o], and PSUM gets
    out[t-block, o-slice] in the natural output orientation.
  - t-blocks are processed in pairs with the contraction (kb) loop
    outermost inside the pair: 8 PSUM banks hold 2x4 accumulation groups
    and the PE can start as soon as the first kb-block of x/w arrives
    instead of waiting for the whole 8 MB weight.
  - Everything (x, w) is SBUF-resident; out streams per t-block.
"""

import numpy as np

P = 128
D = 2048
E = 8
KB = D // P  # 16 contraction blocks
NOS = 4  # 4 moving slices of 512 over the 2048 output dim
NS = D // NOS  # 512

_cache = {}


def _ensure_imports():
    try:
        import concourse.bass  # noqa: F401
    except ImportError:
        import sys

        for p in ("/opt/trn_rl_repo", "/opt/pypackages"):
            if p not in sys.path:
                sys.path.append(p)


def _np_dt(compute_dt):
    if compute_dt == "float16":
        return np.float16
    import ml_dtypes

    return ml_dtypes.bfloat16


def _build(C, compute_dt="float16"):
    """Build + compile the per-core Bass program for capacity C."""
    _ensure_imports()
    import concourse.bacc as bacc
    import concourse.mybir as mybir
    import concourse.tile as tile

    cdt = getattr(mybir.dt, compute_dt)
    TB = C // P  # t-blocks

    nc = bacc.Bacc(None, target_bir_lowering=False, debug=False)
    xt_d = nc.declare_dram_parameter("xt", [D, C], cdt, isOutput=False)
    w_d = nc.declare_dram_parameter("w", [D, D], cdt, isOutput=False)
    out_d = nc.declare_dram_parameter("out", [C, D], mybir.dt.float32, isOutput=True)

    xt_t = xt_d.rearrange("(k p) n -> p k n", p=P)  # [128, 16, C]
    w_t = w_d.rearrange("(k p) o -> p k o", p=P)  # [128, 16, 2048]

    with tile.TileContext(nc) as tc:
        with (
            tc.tile_pool(name="wp", bufs=1) as wp,
            tc.tile_pool(name="xp", bufs=1) as xp,
            tc.tile_pool(name="op", bufs=3) as op,
            tc.tile_pool(name="pp", bufs=8, space="PSUM") as pp,
        ):
            x_sb = []
            w_sb = []
            for kb in range(KB):
                xt_k = xp.tile([P, C], cdt, tag=f"x{kb}")
                nc.sync.dma_start(xt_k[:], xt_t[:, kb, :])
                x_sb.append(xt_k)
                w_k = wp.tile([P, D], cdt, tag=f"w{kb}")
                nc.sync.dma_start(w_k[:], w_t[:, kb, :])
                w_sb.append(w_k)

            for pair0 in range(0, TB, 2):
                tbs = [tb for tb in (pair0, pair0 + 1) if tb < TB]
                ps = {
                    (tb, os): pp.tile(
                        [P, NS], mybir.dt.float32, tag="ps", name=f"ps_{tb}_{os}"
                    )
                    for tb in tbs
                    for os in range(NOS)
                }
                for kb in range(KB):
                    for tb in tbs:
                        for os in range(NOS):
                            nc.tensor.matmul(
                                ps[(tb, os)][:],
                                lhsT=x_sb[kb][:, tb * P : (tb + 1) * P],
                                rhs=w_sb[kb][:, os * NS : (os + 1) * NS],
                                start=(kb == 0),
                                stop=(kb == KB - 1),
                            )
                for tb in tbs:
                    o_sb = op.tile([P, D], mybir.dt.float32, tag="o")
                    for os in range(NOS):
                        nc.vector.tensor_copy(
                            o_sb[:, os * NS : (os + 1) * NS], ps[(tb, os)][:]
                        )
                    nc.sync.dma_start(out_d[tb * P : (tb + 1) * P, :], o_sb[:])
    nc.compile()
    return nc


def _get_nc(C, compute_dt):
    key = (C, compute_dt)
    if key not in _cache:
        _cache[key] = _build(C, compute_dt)
    return _cache[key]


def kernel(tokens, weight, exp_ids, _trace=False, _compute_dt="float16"):
    _ensure_imports()
    from concourse.bass_utils import run_bass_kernel_spmd

    tokens = np.asarray(tokens)
    weight = np.asarray(weight)
    exp_ids = np.asarray(exp_ids)
    T = tokens.shape[0]

    order = np.argsort(exp_ids, kind="stable")
    counts = np.bincount(exp_ids, minlength=E)
    C = max(int(-(-counts.max() // P) * P), NS)

    starts = np.zeros(E + 1, dtype=np.int64)
    np.cumsum(counts, out=starts[1:])

    npdt = _np_dt(_compute_dt)
    tokens_c = tokens.astype(npdt)
    weight_c = weight.astype(npdt)

    in_maps = []
    for e in range(E):
        idx = order[starts[e] : starts[e + 1]]
        xt = np.zeros((D, C), dtype=npdt)
        xt[:, : counts[e]] = tokens_c[idx].T
        in_maps.append({"xt": xt, "w": np.ascontiguousarray(weight_c[e])})

    nc = _get_nc(C, _compute_dt)
    res = run_bass_kernel_spmd(
        nc,
        in_maps,
        core_ids=list(range(E)),
        trace=_trace,
        trace_cores=list(range(E)) if _trace else None,
    )

    out = np.empty((T, D), dtype=np.float32)
    for e in range(E):
        idx = order[starts[e] : starts[e + 1]]
        out[idx] = res.results[e]["out"][: counts[e], :]
    if _trace:
        return out, res
    return out
